# revision 20
# baseline (speedup 1.0000x reference)
"""BCM plasticity kernel for 8 TRN2 NeuronCores.

Strategy: tensor-parallel over out_features (512 per core), x replicated.
Each core computes, fully independently (zero collectives):
  pass 1: out_c = x @ W_c.T            (8192 x 4096) @ (4096 x 512)
          activity_c = mean_b(out_c)   (local: core owns all batch rows
                                        for its 512 output columns)
  mid:    thr_new_c = thr_c + (activity_c^2 - thr_c)/TAU
          post_c = relu(out_c - thr_new_c)   (kept resident in SBUF, bf16)
  pass 2: deltaT_c = x.T-chunks contracted with post_c  -> [IN, 512]
          new_wT_c = wT_c + (LR/B) * deltaT_c
Host pre-transposes/casts inputs so every matmul operand lands in SBUF
with the contraction dim on partitions, and reassembles the full-shape
outputs from the 8 column shards.
"""

import sys
from contextlib import ExitStack

try:
    import concourse.bass  # noqa: F401
except ImportError:  # pragma: no cover
    sys.path.insert(0, "/opt/trn_rl_repo")

import numpy as np
import ml_dtypes

import concourse.bacc as bacc
import concourse.mybir as mybir
import concourse.tile as tile
from concourse.bass_utils import run_bass_kernel_spmd

B, IN, OUT = 8192, 4096, 4096
NCORES = 8
OS = OUT // NCORES          # 512 out features per core
BB = B // 128               # 64 batch blocks
KK = IN // 128              # 32 contraction chunks (pass 1)
IG, JJ = 8, 4               # pass 2: IN split into 8 groups x 4 blocks of 128
LR = 0.001
TAU = 1000.0

# Pass-1 matmul dtype: "bf16" | "f32r" | "f32"
PASS1_DTYPE = "bf16"

BF16 = ml_dtypes.bfloat16

_CACHE = {}


def _build(pass1_dtype):
    f32 = mybir.dt.float32
    bf16 = mybir.dt.bfloat16
    # storage dtype of the pass-1 x/w operands in DRAM/SBUF
    cd_store = bf16 if pass1_dtype == "bf16" else f32
    # dtype stamped on the matmul operand APs
    cd_mm = {
        "bf16": bf16,
        "f32": f32,
        "f32r": mybir.dt.float32r,
    }[pass1_dtype]

    def mmcast(ap):
        return ap.bitcast(cd_mm) if cd_mm != cd_store else ap

    nc = bacc.Bacc(
        "TRN2", target_bir_lowering=False, debug=False, num_devices=NCORES
    )

    f8 = mybir.dt.float8e4
    xt_d = nc.dram_tensor("xt", [BB, 128, KK, 128], cd_store, kind="ExternalInput").ap()
    # pass-2 x operand, fp8 DoubleRow layout: [ig, bchunk, b_in, j, t, i_in]
    # with b = bchunk*256 + t*128 + b_in
    x2_d = nc.dram_tensor(
        "x2", [IG, BB // 4, 128, 2, JJ, 2, 128], f8, kind="ExternalInput"
    ).ap()
    wt_d = nc.dram_tensor("wt", [128, KK, OS], cd_store, kind="ExternalInput").ap()
    wf_d = nc.dram_tensor("wf", [IG, JJ, 128, OS], f32, kind="ExternalInput").ap()
    thr_d = nc.dram_tensor("thr", [1, OS], f32, kind="ExternalInput").ap()

    out_d = nc.dram_tensor("out", [BB, 128, OS], f32, kind="ExternalOutput").ap()
    nw_d = nc.dram_tensor("nw", [IG, JJ, 128, OS], f32, kind="ExternalOutput").ap()
    throut_d = nc.dram_tensor("throut", [1, OS], f32, kind="ExternalOutput").ap()

    with tile.TileContext(nc) as tc:
        with (
            tc.tile_pool(name="const", bufs=1) as constp,
            tc.tile_pool(name="resid", bufs=1) as residp,
            tc.tile_pool(name="xin", bufs=4) as xinp,
            tc.tile_pool(name="stage", bufs=6) as stagep,
            tc.tile_pool(name="x2p", bufs=8) as x2p,
            tc.tile_pool(name="wfp", bufs=10) as wfp,
        ):
            # first batch tile ordered before the weight load so the PE can
            # start as soon as the first 2 MB land
            xt_first = xinp.tile([128, KK, 128], cd_store, tag="xt", name="xt_first")
            nc.sync.dma_start(xt_first[:, :4, :], xt_d[0, :, :4, :])
            nc.sync.dma_start(xt_first[:, 4:, :], xt_d[0, :, 4:, :])
            # resident weights, chunked into 4 tiles so matmul deps are per-chunk
            wt_sbs = []
            for c in range(8):
                wt_c = constp.tile([128, KK // 8, OS], cd_store, name=f"wt_sb{c}")
                nc.sync.dma_start(wt_c[:], wt_d[:, 4 * c : 4 * (c + 1), :])
                wt_sbs.append(wt_c)
            post_sb = residp.tile([128, BB, OS], f8)
            acc = constp.tile([128, OS], f32)
            ones_col = constp.tile([128, 1], f32)
            nc.vector.memset(ones_col[:], 1.0)
            ones_row = constp.tile([1, 128], f32)
            nc.vector.memset(ones_row[:], 1.0)
            thr_sb = constp.tile([1, OS], f32)
            nc.sync.dma_start(thr_sb[:], thr_d[:])
            # thr*(1-1/TAU), computed before pass 1 so the post-pass-1 chain
            # is just square + scalar_tensor_tensor
            thr_scaled = constp.tile([1, OS], f32)
            nc.vector.tensor_scalar_mul(thr_scaled[:], thr_sb[:], 1.0 - 1.0 / TAU)

            # ---------------- pass 1: out = x @ W_c.T ----------------
            ps_stack = ExitStack()
            psp = ps_stack.enter_context(
                tc.tile_pool(name="ps", bufs=4, space="PSUM")
            )
            ps1p = ps_stack.enter_context(
                tc.tile_pool(name="ps1", bufs=1, space="PSUM")
            )
            for bb in range(BB):
                if bb == 0:
                    xt_t = xt_first
                else:
                    xt_t = xinp.tile(
                        [128, KK, 128], cd_store, tag="xt", name=f"xt_{bb}"
                    )
                    nc.sync.dma_start(xt_t[:], xt_d[bb])
                ps = psp.tile([128, OS], f32)
                for k in range(KK):
                    nc.tensor.matmul(
                        ps[:],
                        mmcast(xt_t[:, k, :]),
                        mmcast(wt_sbs[k // 4][:, k % 4, :]),
                        start=(k == 0),
                        stop=(k == KK - 1),
                    )
                st = stagep.tile([128, OS], f32)
                nc.vector.tensor_copy(st[:], ps[:])
                nc.sync.dma_start(out_d[bb], st[:])
                # fp8 copy for pass 2 (ACT engine, casts f32->fp8e4)
                nc.scalar.copy(post_sb[:, bb, :], st[:])
                # batch-sum accumulation for activity (DVE, f32)
                if bb == 0:
                    nc.vector.tensor_copy(acc[:], st[:])
                else:
                    nc.vector.tensor_add(acc[:], acc[:], st[:])

            # ---------------- mid: activity / threshold ----------------
            act_ps = ps1p.tile([1, OS], f32)
            nc.tensor.matmul(act_ps[:], ones_col[:], acc[:], start=True, stop=True)
            # thr_new = thr*(1-1/TAU) + (act_ps/B)^2 / TAU
            sq = constp.tile([1, OS], f32)
            nc.scalar.square(sq[:], act_ps[:])
            thr_new = constp.tile([1, OS], f32)
            nc.vector.scalar_tensor_tensor(
                out=thr_new[:],
                in0=sq[:],
                scalar=float(1.0 / (B * B * TAU)),
                in1=thr_scaled[:],
                op0=mybir.AluOpType.mult,
                op1=mybir.AluOpType.add,
            )
            nc.sync.dma_start(throut_d[:], thr_new[:])

            # broadcast thr_new across 128 partitions via K=1 matmul
            bc_ps = ps1p.tile([128, OS], f32, tag="bcps")
            nc.tensor.matmul(bc_ps[:], ones_row[:], thr_new[:], start=True, stop=True)
            thr_bc = constp.tile([128, OS], bf16)
            nc.scalar.copy(thr_bc[:], bc_ps[:])

            # post = relu(out - thr_new), in place on the resident bf16 copy
            for bb in range(BB):
                nc.vector.tensor_sub(
                    post_sb[:, bb, :], post_sb[:, bb, :], thr_bc[:]
                )
                nc.vector.tensor_relu(post_sb[:, bb, :], post_sb[:, bb, :])

            # ---------------- pass 2: deltaT = sum_b x[b,i] * post[b,o] ----
            ps_stack.close()  # release pass-1 PSUM banks
            with tc.tile_pool(name="ps2", bufs=2, space="PSUM") as ps2p:
                for ig in range(IG):
                    ps2 = []
                    for j in range(JJ):
                        ps2.append(ps2p.tile([128, OS], f32, tag=f"d{j}", name=f"ps2_{ig}_{j}"))
                    # prefetch this ig's weight tiles before the batch sweep
                    wf_ts = []
                    for j in range(JJ):
                        wf_t = wfp.tile([128, OS], f32, tag="wf", name=f"wf_{ig}_{j}")
                        nc.sync.dma_start(wf_t[:], wf_d[ig, j])
                        wf_ts.append(wf_t)
                    for cc in range(BB // 4):
                        x2_t = x2p.tile([128, 2, JJ, 2, 128], f8)
                        nc.gpsimd.dma_start(x2_t[:], x2_d[ig, cc])
                        for c2 in range(2):
                            bc = 2 * cc + c2
                            for j in range(JJ):
                                nc.tensor.matmul(
                                    ps2[j][:],
                                    x2_t[:, c2, j, :, :],
                                    post_sb[:, 2 * bc : 2 * bc + 2, :],
                                    start=(bc == 0),
                                    stop=(bc == BB // 2 - 1),
                                    perf_mode=mybir.MatmulPerfMode.DoubleRow,
                                )
                    for j in range(JJ):
                        wf_t = wf_ts[j]
                        st2 = stagep.tile([128, OS], f32)
                        nc.vector.scalar_tensor_tensor(
                            out=st2[:],
                            in0=ps2[j][:],
                            scalar=float(LR / B),
                            in1=wf_t[:],
                            op0=mybir.AluOpType.mult,
                            op1=mybir.AluOpType.add,
                        )
                        nc.sync.dma_start(nw_d[ig, j], st2[:])

    nc.compile()
    return nc


def _get_compiled(pass1_dtype=None):
    key = pass1_dtype or PASS1_DTYPE
    if key not in _CACHE:
        _CACHE[key] = _build(key)
    return _CACHE[key]


def _prep_in_maps(x, weight, threshold, pass1_dtype=None):
    pd = pass1_dtype or PASS1_DTYPE
    cd_np = BF16 if pd == "bf16" else np.float32

    x = np.asarray(x, dtype=np.float32)
    weight = np.asarray(weight, dtype=np.float32)
    threshold = np.asarray(threshold, dtype=np.float32)

    xc = x.astype(cd_np)
    # xt[bb, i_in, k, b_in] = x[bb*128+b_in, k*128+i_in]
    xt = np.ascontiguousarray(
        xc.reshape(BB, 128, KK, 128).transpose(0, 3, 2, 1)
    )
    x8 = x.astype(ml_dtypes.float8_e4m3)
    # x2[ig, cc, b_in, c2, j, t, i_in] = x[(cc*2+c2)*256 + t*128 + b_in,
    #                                      (ig*JJ+j)*128 + i_in]
    x2 = np.ascontiguousarray(
        x8.reshape(BB // 4, 2, 2, 128, IG, JJ, 128).transpose(4, 0, 3, 1, 5, 2, 6)
    )

    in_maps = []
    for c in range(NCORES):
        wsh = weight[c * OS : (c + 1) * OS]  # (512, 4096)
        # wt[i_in, k, o] = wsh[o, k*128+i_in]
        wt = np.ascontiguousarray(
            wsh.astype(cd_np).reshape(OS, KK, 128).transpose(2, 1, 0)
        )
        # wf[ig, j, i_in, o] = wsh[o, ig*1024+j*128+i_in]
        wf = np.ascontiguousarray(
            wsh.reshape(OS, IG, JJ, 128).transpose(1, 2, 3, 0)
        )
        thr = np.ascontiguousarray(
            threshold[c * OS : (c + 1) * OS].reshape(1, OS)
        )
        in_maps.append({"xt": xt, "x2": x2, "wt": wt, "wf": wf, "thr": thr})
    return in_maps


def _assemble(results):
    outs = [np.asarray(r["out"], dtype=np.float32).reshape(B, OS) for r in results]
    output = np.concatenate(outs, axis=1)
    thr_new = np.concatenate(
        [np.asarray(r["throut"], dtype=np.float32).reshape(OS) for r in results]
    )
    nws = [
        np.asarray(r["nw"], dtype=np.float32).transpose(3, 0, 1, 2).reshape(OS, IN)
        for r in results
    ]
    new_weight = np.concatenate(nws, axis=0)
    return output, thr_new, new_weight


def _run(x, weight, threshold, pass1_dtype=None, trace=False):
    nc = _get_compiled(pass1_dtype)
    in_maps = _prep_in_maps(x, weight, threshold, pass1_dtype)
    res = run_bass_kernel_spmd(
        nc, in_maps, core_ids=list(range(NCORES)), trace=trace
    )
    return _assemble(res.results), res


def kernel(x, weight, threshold):
    (output, thr_new, new_weight), _ = _run(x, weight, threshold)
    return output, thr_new, new_weight


# revision 22
# speedup vs baseline: 1.1915x; 1.1915x over previous
"""BCM plasticity kernel for 8 TRN2 NeuronCores.

Strategy: tensor-parallel over out_features (512 per core), x replicated.
Each core computes, fully independently (zero collectives):
  pass 1: out_c = x @ W_c.T            (8192 x 4096) @ (4096 x 512)
          activity_c = mean_b(out_c)   (local: core owns all batch rows
                                        for its 512 output columns)
  mid:    thr_new_c = thr_c + (activity_c^2 - thr_c)/TAU
          post_c = relu(out_c - thr_new_c)   (kept resident in SBUF, bf16)
  pass 2: deltaT_c = x.T-chunks contracted with post_c  -> [IN, 512]
          new_wT_c = wT_c + (LR/B) * deltaT_c
Host pre-transposes/casts inputs so every matmul operand lands in SBUF
with the contraction dim on partitions, and reassembles the full-shape
outputs from the 8 column shards.
"""

import sys
from contextlib import ExitStack

try:
    import concourse.bass  # noqa: F401
except ImportError:  # pragma: no cover
    sys.path.insert(0, "/opt/trn_rl_repo")

import numpy as np
import ml_dtypes

import concourse.bacc as bacc
import concourse.mybir as mybir
import concourse.tile as tile
from concourse.bass_utils import run_bass_kernel_spmd

B, IN, OUT = 8192, 4096, 4096
NCORES = 8
OS = OUT // NCORES          # 512 out features per core
BB = B // 128               # 64 batch blocks
KK = IN // 128              # 32 contraction chunks (pass 1)
IG, JJ = 8, 4               # pass 2: IN split into 8 groups x 4 blocks of 128
LR = 0.001
TAU = 1000.0

# Pass-1 matmul dtype: "bf16" | "f32r" | "f32"
PASS1_DTYPE = "bf16"

BF16 = ml_dtypes.bfloat16

_CACHE = {}


def _build(pass1_dtype):
    f32 = mybir.dt.float32
    bf16 = mybir.dt.bfloat16
    # storage dtype of the pass-1 x/w operands in DRAM/SBUF
    cd_store = bf16 if pass1_dtype == "bf16" else f32
    # dtype stamped on the matmul operand APs
    cd_mm = {
        "bf16": bf16,
        "f32": f32,
        "f32r": mybir.dt.float32r,
    }[pass1_dtype]

    def mmcast(ap):
        return ap.bitcast(cd_mm) if cd_mm != cd_store else ap

    nc = bacc.Bacc(
        "TRN2", target_bir_lowering=False, debug=False, num_devices=NCORES
    )

    f8 = mybir.dt.float8e4
    xt_d = nc.dram_tensor("xt", [BB, 128, KK, 128], cd_store, kind="ExternalInput").ap()
    # pass-2 x operand, fp8 DoubleRow layout: [ig, bchunk, b_in, j, t, i_in]
    # with b = bchunk*256 + t*128 + b_in
    x2_d = nc.dram_tensor(
        "x2", [IG, BB // 4, 128, 2, JJ, 2, 128], f8, kind="ExternalInput"
    ).ap()
    wt_d = nc.dram_tensor("wt", [128, KK, OS], cd_store, kind="ExternalInput").ap()
    wf_d = nc.dram_tensor("wf", [IG, JJ, 128, OS], f32, kind="ExternalInput").ap()
    thr_d = nc.dram_tensor("thr", [1, OS], f32, kind="ExternalInput").ap()

    out_d = nc.dram_tensor("out", [BB, 128, OS], f32, kind="ExternalOutput").ap()
    nw_d = nc.dram_tensor("nw", [IG, JJ, 128, OS], f32, kind="ExternalOutput").ap()
    throut_d = nc.dram_tensor("throut", [1, OS], f32, kind="ExternalOutput").ap()

    with tile.TileContext(nc) as tc:
        with (
            tc.tile_pool(name="const", bufs=1) as constp,
            tc.tile_pool(name="resid", bufs=1) as residp,
            tc.tile_pool(name="xin", bufs=4) as xinp,
            tc.tile_pool(name="stage", bufs=6) as stagep,
            tc.tile_pool(name="x2p", bufs=12) as x2p,
            tc.tile_pool(name="wfp", bufs=10) as wfp,
        ):
            # first batch tile ordered before the weight load so the PE can
            # start as soon as the first 2 MB land
            xt_first = xinp.tile([128, KK, 128], cd_store, tag="xt", name="xt_first")
            nc.sync.dma_start(xt_first[:, :4, :], xt_d[0, :, :4, :])
            nc.sync.dma_start(xt_first[:, 4:, :], xt_d[0, :, 4:, :])
            # resident weights, chunked into 4 tiles so matmul deps are per-chunk
            wt_sbs = []
            for c in range(8):
                wt_c = constp.tile([128, KK // 8, OS], cd_store, name=f"wt_sb{c}")
                nc.sync.dma_start(wt_c[:], wt_d[:, 4 * c : 4 * (c + 1), :])
                wt_sbs.append(wt_c)
            post_sb = residp.tile([128, BB, OS], f8)
            acc = constp.tile([128, OS], f32)
            ones_col = constp.tile([128, 1], f32)
            nc.vector.memset(ones_col[:], 1.0)
            ones_row = constp.tile([1, 128], f32)
            nc.vector.memset(ones_row[:], 1.0)
            thr_sb = constp.tile([1, OS], f32)
            nc.sync.dma_start(thr_sb[:], thr_d[:])
            # thr*(1-1/TAU), computed before pass 1 so the post-pass-1 chain
            # is just square + scalar_tensor_tensor
            thr_scaled = constp.tile([1, OS], f32)
            nc.vector.tensor_scalar_mul(thr_scaled[:], thr_sb[:], 1.0 - 1.0 / TAU)

            # ---------------- pass 1: out = x @ W_c.T ----------------
            ps_stack = ExitStack()
            psp = ps_stack.enter_context(
                tc.tile_pool(name="ps", bufs=4, space="PSUM")
            )
            ps1p = ps_stack.enter_context(
                tc.tile_pool(name="ps1", bufs=1, space="PSUM")
            )
            for bb in range(BB):
                if bb == 0:
                    xt_t = xt_first
                else:
                    xt_t = xinp.tile(
                        [128, KK, 128], cd_store, tag="xt", name=f"xt_{bb}"
                    )
                    nc.sync.dma_start(xt_t[:], xt_d[bb])
                ps = psp.tile([128, OS], f32)
                for k in range(KK):
                    nc.tensor.matmul(
                        ps[:],
                        mmcast(xt_t[:, k, :]),
                        mmcast(wt_sbs[k // 4][:, k % 4, :]),
                        start=(k == 0),
                        stop=(k == KK - 1),
                    )
                st = stagep.tile([128, OS], f32)
                nc.vector.tensor_copy(st[:], ps[:])
                nc.sync.dma_start(out_d[bb], st[:])
                # fp8 copy for pass 2 (ACT engine, casts f32->fp8e4)
                nc.scalar.copy(post_sb[:, bb, :], st[:])
                # batch-sum accumulation for activity (DVE, f32)
                if bb == 0:
                    nc.vector.tensor_copy(acc[:], st[:])
                else:
                    nc.vector.tensor_add(acc[:], acc[:], st[:])

            # ---------------- mid: activity / threshold ----------------
            act_ps = ps1p.tile([1, OS], f32)
            nc.tensor.matmul(act_ps[:], ones_col[:], acc[:], start=True, stop=True)
            # thr_new = thr*(1-1/TAU) + (act_ps/B)^2 / TAU
            sq = constp.tile([1, OS], f32)
            nc.scalar.square(sq[:], act_ps[:])
            thr_new = constp.tile([1, OS], f32)
            nc.vector.scalar_tensor_tensor(
                out=thr_new[:],
                in0=sq[:],
                scalar=float(1.0 / (B * B * TAU)),
                in1=thr_scaled[:],
                op0=mybir.AluOpType.mult,
                op1=mybir.AluOpType.add,
            )
            nc.sync.dma_start(throut_d[:], thr_new[:])

            # broadcast thr_new across 128 partitions via K=1 matmul
            bc_ps = ps1p.tile([128, OS], f32, tag="bcps")
            nc.tensor.matmul(bc_ps[:], ones_row[:], thr_new[:], start=True, stop=True)
            thr_bc = constp.tile([128, OS], bf16)
            nc.vector.tensor_copy(thr_bc[:], bc_ps[:])

            # post = relu(out - thr_new), in place on the resident bf16 copy
            for bb in range(BB):
                nc.vector.tensor_sub(
                    post_sb[:, bb, :], post_sb[:, bb, :], thr_bc[:]
                )
                nc.vector.tensor_relu(post_sb[:, bb, :], post_sb[:, bb, :])

            # ---------------- pass 2: deltaT = sum_b x[b,i] * post[b,o] ----
            ps_stack.close()  # release pass-1 PSUM banks
            with tc.tile_pool(name="ps2", bufs=2, space="PSUM") as ps2p:
                for ig in range(IG):
                    ps2 = []
                    for j in range(JJ):
                        ps2.append(ps2p.tile([128, OS], f32, tag=f"d{j}", name=f"ps2_{ig}_{j}"))
                    # prefetch this ig's weight tiles before the batch sweep
                    wf_ts = []
                    for j in range(JJ):
                        wf_t = wfp.tile([128, OS], f32, tag="wf", name=f"wf_{ig}_{j}")
                        nc.sync.dma_start(wf_t[:], wf_d[ig, j])
                        wf_ts.append(wf_t)
                    for cc in range(BB // 4):
                        x2_t = x2p.tile([128, 2, JJ, 2, 128], f8)
                        nc.gpsimd.dma_start(x2_t[:], x2_d[ig, cc])
                        for c2 in range(2):
                            bc = 2 * cc + c2
                            for j in range(JJ):
                                nc.tensor.matmul(
                                    ps2[j][:],
                                    x2_t[:, c2, j, :, :],
                                    post_sb[:, 2 * bc : 2 * bc + 2, :],
                                    start=(bc == 0),
                                    stop=(bc == BB // 2 - 1),
                                    perf_mode=mybir.MatmulPerfMode.DoubleRow,
                                )
                    for j in range(JJ):
                        wf_t = wf_ts[j]
                        st2 = stagep.tile([128, OS], f32)
                        nc.vector.scalar_tensor_tensor(
                            out=st2[:],
                            in0=ps2[j][:],
                            scalar=float(LR / B),
                            in1=wf_t[:],
                            op0=mybir.AluOpType.mult,
                            op1=mybir.AluOpType.add,
                        )
                        nc.sync.dma_start(nw_d[ig, j], st2[:])

    nc.compile()
    return nc


def _get_compiled(pass1_dtype=None):
    key = pass1_dtype or PASS1_DTYPE
    if key not in _CACHE:
        _CACHE[key] = _build(key)
    return _CACHE[key]


def _prep_in_maps(x, weight, threshold, pass1_dtype=None):
    pd = pass1_dtype or PASS1_DTYPE
    cd_np = BF16 if pd == "bf16" else np.float32

    x = np.asarray(x, dtype=np.float32)
    weight = np.asarray(weight, dtype=np.float32)
    threshold = np.asarray(threshold, dtype=np.float32)

    xc = x.astype(cd_np)
    # xt[bb, i_in, k, b_in] = x[bb*128+b_in, k*128+i_in]
    xt = np.ascontiguousarray(
        xc.reshape(BB, 128, KK, 128).transpose(0, 3, 2, 1)
    )
    x8 = x.astype(ml_dtypes.float8_e4m3)
    # x2[ig, cc, b_in, c2, j, t, i_in] = x[(cc*2+c2)*256 + t*128 + b_in,
    #                                      (ig*JJ+j)*128 + i_in]
    x2 = np.ascontiguousarray(
        x8.reshape(BB // 4, 2, 2, 128, IG, JJ, 128).transpose(4, 0, 3, 1, 5, 2, 6)
    )

    in_maps = []
    for c in range(NCORES):
        wsh = weight[c * OS : (c + 1) * OS]  # (512, 4096)
        # wt[i_in, k, o] = wsh[o, k*128+i_in]
        wt = np.ascontiguousarray(
            wsh.astype(cd_np).reshape(OS, KK, 128).transpose(2, 1, 0)
        )
        # wf[ig, j, i_in, o] = wsh[o, ig*1024+j*128+i_in]
        wf = np.ascontiguousarray(
            wsh.reshape(OS, IG, JJ, 128).transpose(1, 2, 3, 0)
        )
        thr = np.ascontiguousarray(
            threshold[c * OS : (c + 1) * OS].reshape(1, OS)
        )
        in_maps.append({"xt": xt, "x2": x2, "wt": wt, "wf": wf, "thr": thr})
    return in_maps


def _assemble(results):
    outs = [np.asarray(r["out"], dtype=np.float32).reshape(B, OS) for r in results]
    output = np.concatenate(outs, axis=1)
    thr_new = np.concatenate(
        [np.asarray(r["throut"], dtype=np.float32).reshape(OS) for r in results]
    )
    nws = [
        np.asarray(r["nw"], dtype=np.float32).transpose(3, 0, 1, 2).reshape(OS, IN)
        for r in results
    ]
    new_weight = np.concatenate(nws, axis=0)
    return output, thr_new, new_weight


def _run(x, weight, threshold, pass1_dtype=None, trace=False):
    nc = _get_compiled(pass1_dtype)
    in_maps = _prep_in_maps(x, weight, threshold, pass1_dtype)
    res = run_bass_kernel_spmd(
        nc, in_maps, core_ids=list(range(NCORES)), trace=trace
    )
    return _assemble(res.results), res


def kernel(x, weight, threshold):
    (output, thr_new, new_weight), _ = _run(x, weight, threshold)
    return output, thr_new, new_weight


# revision 23
# speedup vs baseline: 1.1944x; 1.0025x over previous
"""BCM plasticity kernel for 8 TRN2 NeuronCores.

Strategy: tensor-parallel over out_features (512 per core), x replicated.
Each core computes, fully independently (zero collectives):
  pass 1: out_c = x @ W_c.T            (8192 x 4096) @ (4096 x 512)
          activity_c = mean_b(out_c)   (local: core owns all batch rows
                                        for its 512 output columns)
  mid:    thr_new_c = thr_c + (activity_c^2 - thr_c)/TAU
          post_c = relu(out_c - thr_new_c)   (kept resident in SBUF, bf16)
  pass 2: deltaT_c = x.T-chunks contracted with post_c  -> [IN, 512]
          new_wT_c = wT_c + (LR/B) * deltaT_c
Host pre-transposes/casts inputs so every matmul operand lands in SBUF
with the contraction dim on partitions, and reassembles the full-shape
outputs from the 8 column shards.
"""

import sys
from contextlib import ExitStack

try:
    import concourse.bass  # noqa: F401
except ImportError:  # pragma: no cover
    sys.path.insert(0, "/opt/trn_rl_repo")

import numpy as np
import ml_dtypes

import concourse.bacc as bacc
import concourse.mybir as mybir
import concourse.tile as tile
from concourse.bass_utils import run_bass_kernel_spmd

B, IN, OUT = 8192, 4096, 4096
NCORES = 8
OS = OUT // NCORES          # 512 out features per core
BB = B // 128               # 64 batch blocks
KK = IN // 128              # 32 contraction chunks (pass 1)
IG, JJ = 8, 4               # pass 2: IN split into 8 groups x 4 blocks of 128
LR = 0.001
TAU = 1000.0

# Pass-1 matmul dtype: "bf16" | "f32r" | "f32"
PASS1_DTYPE = "bf16"

BF16 = ml_dtypes.bfloat16

_CACHE = {}


def _build(pass1_dtype):
    f32 = mybir.dt.float32
    bf16 = mybir.dt.bfloat16
    # storage dtype of the pass-1 x/w operands in DRAM/SBUF
    cd_store = bf16 if pass1_dtype == "bf16" else f32
    # dtype stamped on the matmul operand APs
    cd_mm = {
        "bf16": bf16,
        "f32": f32,
        "f32r": mybir.dt.float32r,
    }[pass1_dtype]

    def mmcast(ap):
        return ap.bitcast(cd_mm) if cd_mm != cd_store else ap

    nc = bacc.Bacc(
        "TRN2", target_bir_lowering=False, debug=False, num_devices=NCORES
    )

    f8 = mybir.dt.float8e4
    xt_d = nc.dram_tensor("xt", [BB, 128, KK, 128], cd_store, kind="ExternalInput").ap()
    # pass-2 x operand, fp8 DoubleRow layout: [ig, bchunk, b_in, j, t, i_in]
    # with b = bchunk*256 + t*128 + b_in
    x2_d = nc.dram_tensor(
        "x2", [IG, BB // 4, 128, 2, JJ, 2, 128], f8, kind="ExternalInput"
    ).ap()
    wt_d = nc.dram_tensor("wt", [128, KK, OS], cd_store, kind="ExternalInput").ap()
    wf_d = nc.dram_tensor("wf", [IG, JJ, 128, OS], f32, kind="ExternalInput").ap()
    thr_d = nc.dram_tensor("thr", [1, OS], f32, kind="ExternalInput").ap()

    out_d = nc.dram_tensor("out", [BB, 128, OS], f32, kind="ExternalOutput").ap()
    nw_d = nc.dram_tensor("nw", [IG, JJ, 128, OS], f32, kind="ExternalOutput").ap()
    throut_d = nc.dram_tensor("throut", [1, OS], f32, kind="ExternalOutput").ap()

    with tile.TileContext(nc) as tc:
        with (
            tc.tile_pool(name="const", bufs=1) as constp,
            tc.tile_pool(name="resid", bufs=1) as residp,
            tc.tile_pool(name="xin", bufs=6) as xinp,
            tc.tile_pool(name="stage", bufs=6) as stagep,
            tc.tile_pool(name="x2p", bufs=12) as x2p,
            tc.tile_pool(name="wfp", bufs=10) as wfp,
        ):
            # first batch tile ordered before the weight load so the PE can
            # start as soon as the first 2 MB land
            xt_first = xinp.tile([128, KK, 128], cd_store, tag="xt", name="xt_first")
            nc.sync.dma_start(xt_first[:, :4, :], xt_d[0, :, :4, :])
            nc.sync.dma_start(xt_first[:, 4:, :], xt_d[0, :, 4:, :])
            # resident weights, chunked into 4 tiles so matmul deps are per-chunk
            wt_sbs = []
            for c in range(8):
                wt_c = constp.tile([128, KK // 8, OS], cd_store, name=f"wt_sb{c}")
                nc.sync.dma_start(wt_c[:], wt_d[:, 4 * c : 4 * (c + 1), :])
                wt_sbs.append(wt_c)
            post_sb = residp.tile([128, BB, OS], f8)
            acc = constp.tile([128, OS], f32)
            ones_col = constp.tile([128, 1], f32)
            nc.vector.memset(ones_col[:], 1.0)
            ones_row = constp.tile([1, 128], f32)
            nc.vector.memset(ones_row[:], 1.0)
            thr_sb = constp.tile([1, OS], f32)
            nc.sync.dma_start(thr_sb[:], thr_d[:])
            # thr*(1-1/TAU), computed before pass 1 so the post-pass-1 chain
            # is just square + scalar_tensor_tensor
            thr_scaled = constp.tile([1, OS], f32)
            nc.vector.tensor_scalar_mul(thr_scaled[:], thr_sb[:], 1.0 - 1.0 / TAU)

            # ---------------- pass 1: out = x @ W_c.T ----------------
            ps_stack = ExitStack()
            psp = ps_stack.enter_context(
                tc.tile_pool(name="ps", bufs=4, space="PSUM")
            )
            ps1p = ps_stack.enter_context(
                tc.tile_pool(name="ps1", bufs=1, space="PSUM")
            )
            for bb in range(BB):
                if bb == 0:
                    xt_t = xt_first
                else:
                    xt_t = xinp.tile(
                        [128, KK, 128], cd_store, tag="xt", name=f"xt_{bb}"
                    )
                    nc.sync.dma_start(xt_t[:], xt_d[bb])
                ps = psp.tile([128, OS], f32)
                for k in range(KK):
                    nc.tensor.matmul(
                        ps[:],
                        mmcast(xt_t[:, k, :]),
                        mmcast(wt_sbs[k // 4][:, k % 4, :]),
                        start=(k == 0),
                        stop=(k == KK - 1),
                    )
                st = stagep.tile([128, OS], f32)
                nc.vector.tensor_copy(st[:], ps[:])
                nc.sync.dma_start(out_d[bb], st[:])
                # fp8 copy for pass 2 (ACT engine, casts f32->fp8e4)
                nc.scalar.copy(post_sb[:, bb, :], st[:])
                # batch-sum accumulation for activity (DVE, f32)
                if bb == 0:
                    nc.vector.tensor_copy(acc[:], st[:])
                else:
                    nc.vector.tensor_add(acc[:], acc[:], st[:])

            # ---------------- mid: activity / threshold ----------------
            act_ps = ps1p.tile([1, OS], f32)
            nc.tensor.matmul(act_ps[:], ones_col[:], acc[:], start=True, stop=True)
            # thr_new = thr*(1-1/TAU) + (act_ps/B)^2 / TAU
            act_sb = constp.tile([1, OS], f32)
            nc.vector.tensor_copy(act_sb[:], act_ps[:])
            sq = constp.tile([1, OS], f32)
            nc.vector.tensor_mul(sq[:], act_ps[:], act_sb[:])
            thr_new = constp.tile([1, OS], f32)
            nc.vector.scalar_tensor_tensor(
                out=thr_new[:],
                in0=sq[:],
                scalar=float(1.0 / (B * B * TAU)),
                in1=thr_scaled[:],
                op0=mybir.AluOpType.mult,
                op1=mybir.AluOpType.add,
            )
            nc.sync.dma_start(throut_d[:], thr_new[:])

            # broadcast thr_new across 128 partitions via K=1 matmul
            bc_ps = ps1p.tile([128, OS], f32, tag="bcps")
            nc.tensor.matmul(bc_ps[:], ones_row[:], thr_new[:], start=True, stop=True)
            thr_bc = constp.tile([128, OS], bf16)
            nc.vector.tensor_copy(thr_bc[:], bc_ps[:])

            # post = relu(out - thr_new), in place on the resident bf16 copy
            for bb in range(BB):
                nc.vector.tensor_sub(
                    post_sb[:, bb, :], post_sb[:, bb, :], thr_bc[:]
                )
                nc.vector.tensor_relu(post_sb[:, bb, :], post_sb[:, bb, :])

            # ---------------- pass 2: deltaT = sum_b x[b,i] * post[b,o] ----
            ps_stack.close()  # release pass-1 PSUM banks
            with tc.tile_pool(name="ps2", bufs=2, space="PSUM") as ps2p:
                for ig in range(IG):
                    ps2 = []
                    for j in range(JJ):
                        ps2.append(ps2p.tile([128, OS], f32, tag=f"d{j}", name=f"ps2_{ig}_{j}"))
                    # prefetch this ig's weight tiles before the batch sweep
                    wf_ts = []
                    for j in range(JJ):
                        wf_t = wfp.tile([128, OS], f32, tag="wf", name=f"wf_{ig}_{j}")
                        nc.sync.dma_start(wf_t[:], wf_d[ig, j])
                        wf_ts.append(wf_t)
                    for cc in range(BB // 4):
                        x2_t = x2p.tile([128, 2, JJ, 2, 128], f8)
                        nc.gpsimd.dma_start(x2_t[:], x2_d[ig, cc])
                        for c2 in range(2):
                            bc = 2 * cc + c2
                            for j in range(JJ):
                                nc.tensor.matmul(
                                    ps2[j][:],
                                    x2_t[:, c2, j, :, :],
                                    post_sb[:, 2 * bc : 2 * bc + 2, :],
                                    start=(bc == 0),
                                    stop=(bc == BB // 2 - 1),
                                    perf_mode=mybir.MatmulPerfMode.DoubleRow,
                                )
                    for j in range(JJ):
                        wf_t = wf_ts[j]
                        st2 = stagep.tile([128, OS], f32)
                        nc.vector.scalar_tensor_tensor(
                            out=st2[:],
                            in0=ps2[j][:],
                            scalar=float(LR / B),
                            in1=wf_t[:],
                            op0=mybir.AluOpType.mult,
                            op1=mybir.AluOpType.add,
                        )
                        nc.sync.dma_start(nw_d[ig, j], st2[:])

    nc.compile()
    return nc


def _get_compiled(pass1_dtype=None):
    key = pass1_dtype or PASS1_DTYPE
    if key not in _CACHE:
        _CACHE[key] = _build(key)
    return _CACHE[key]


def _prep_in_maps(x, weight, threshold, pass1_dtype=None):
    pd = pass1_dtype or PASS1_DTYPE
    cd_np = BF16 if pd == "bf16" else np.float32

    x = np.asarray(x, dtype=np.float32)
    weight = np.asarray(weight, dtype=np.float32)
    threshold = np.asarray(threshold, dtype=np.float32)

    xc = x.astype(cd_np)
    # xt[bb, i_in, k, b_in] = x[bb*128+b_in, k*128+i_in]
    xt = np.ascontiguousarray(
        xc.reshape(BB, 128, KK, 128).transpose(0, 3, 2, 1)
    )
    x8 = x.astype(ml_dtypes.float8_e4m3)
    # x2[ig, cc, b_in, c2, j, t, i_in] = x[(cc*2+c2)*256 + t*128 + b_in,
    #                                      (ig*JJ+j)*128 + i_in]
    x2 = np.ascontiguousarray(
        x8.reshape(BB // 4, 2, 2, 128, IG, JJ, 128).transpose(4, 0, 3, 1, 5, 2, 6)
    )

    in_maps = []
    for c in range(NCORES):
        wsh = weight[c * OS : (c + 1) * OS]  # (512, 4096)
        # wt[i_in, k, o] = wsh[o, k*128+i_in]
        wt = np.ascontiguousarray(
            wsh.astype(cd_np).reshape(OS, KK, 128).transpose(2, 1, 0)
        )
        # wf[ig, j, i_in, o] = wsh[o, ig*1024+j*128+i_in]
        wf = np.ascontiguousarray(
            wsh.reshape(OS, IG, JJ, 128).transpose(1, 2, 3, 0)
        )
        thr = np.ascontiguousarray(
            threshold[c * OS : (c + 1) * OS].reshape(1, OS)
        )
        in_maps.append({"xt": xt, "x2": x2, "wt": wt, "wf": wf, "thr": thr})
    return in_maps


def _assemble(results):
    outs = [np.asarray(r["out"], dtype=np.float32).reshape(B, OS) for r in results]
    output = np.concatenate(outs, axis=1)
    thr_new = np.concatenate(
        [np.asarray(r["throut"], dtype=np.float32).reshape(OS) for r in results]
    )
    nws = [
        np.asarray(r["nw"], dtype=np.float32).transpose(3, 0, 1, 2).reshape(OS, IN)
        for r in results
    ]
    new_weight = np.concatenate(nws, axis=0)
    return output, thr_new, new_weight


def _run(x, weight, threshold, pass1_dtype=None, trace=False):
    nc = _get_compiled(pass1_dtype)
    in_maps = _prep_in_maps(x, weight, threshold, pass1_dtype)
    res = run_bass_kernel_spmd(
        nc, in_maps, core_ids=list(range(NCORES)), trace=trace
    )
    return _assemble(res.results), res


def kernel(x, weight, threshold):
    (output, thr_new, new_weight), _ = _run(x, weight, threshold)
    return output, thr_new, new_weight
